# revision 19
# baseline (speedup 1.0000x reference)
"""Trainium2 Bass kernel for MultiHeadSelfAttentionModelV1.

Model (per batch row):
    e   = emb_table[x]                      # [S, E]
    Q/K/V = e @ W* + b*                     # [S, E], split into H heads of Dh
    P_h = softmax(Q_h K_h^T / sqrt(Dh))
    ctx = concat_h(P_h V_h) @ Wo + bo       # [S, E]
    out = max_tokens(ctx) @ Wc + bc         # [OUT]

Sharding: pure data parallel over batch; B == n_cores == 8, one row per core.

Measured: ~310 us HW (vs 420 us baseline), rel err 9.3e-3 (gate 2e-2).

DMA-ordering note: the softmax-denominator fanout is a stride-0
broadcast DMA reading a DRAM bounce written by another DMA. With both
on the sync queue this flaked (~1/14 runs): the framework elides the
completion semaphore for same-queue pairs while DMA hardware can
reorder completions within a queue. The read is therefore issued from
the GPSIMD queue — the built program then carries an explicit
wait:S[DMAHW*]>= completion-semaphore edge (verified by instruction
inspection), making the write -> read ordering architectural.

Key design choices (numerics validated in numpy sim, sim == HW err):
 - emb table bf16 in DRAM; token gather via indirect DMA, PE-transpose in
   bf16, ONE strided eviction per token tile into a single eT supertile.
 - Q/K/V projections bf16 (fp8 DoubleRow was tried: only -4 us, err
   1.6e-2 — rejected). Bias folds into the PSUM eviction (ACT
   activation-Identity-bias for Q, DVE tensor_scalar_add for K).
 - V is evicted to fp8(e4m3) pair tiles va2[p][128 tok, 8 heads x 2
   k-subtiles x 80] laid out [V_h (64) | 1 | pad15] (DoubleRow ldweights
   needs subtile step % 16 == 0); the ones column makes the PV matmul
   accumulate the softmax denominator in PSUM row 64 for free.
 - softmax exp is ONE fused op per [128,1024] score tile:
   u8 = s*(1/ln2) + 55.35, bitcast u8 -> e4m3 IS exp(s/8) (Schraudolph in
   fp8 bit space). The quantization bias cancels in the softmax
   denominator because the denominator sums the same quantized P. Runs on
   ACT (activation Copy w/ scale+bias -> u8) or DVE (tensor_scalar
   mult,add -> u8), ONE engine per (j, head-pair) unit so every softmax
   row sees a single convert-rounding mode. GPSIMD cannot read PSUM on
   TRN2, so only these two engines can exp; ~145 us/engine is the floor.
 - PV runs fp8 DoubleRow: two 128-token k-tiles per matmul = 2x fewer PE
   streaming cycles. ctx accumulates [65, 512] fp32 in one PSUM bank.
 - Two-pass heads per unit: the chunk loop accumulates only head-e; the
   SBUF pt tiles are replayed through 8 more DoubleRow matmuls for head-o.
   This halves live ctx banks (2 instead of 4), buying a THIRD stile slot
   (sps bufs=3 x 2 banks + ctx 2 banks = all 8 PSUM banks) so scores(k+1)
   overlaps exp(k) instead of serializing on the slot.
 - Phase C is software-pipelined: two (j, head-pair) units in flight,
   ACT/DVE alternating, interleaved at chunk granularity. Q/K projection
   blocks are emitted lazily as unit prerequisites (their PE work fills
   exp-bound slack).
 - Normalization: ctx evicted on the unit's OWN exp engine (no priority
   inversion), both denominator rows land contiguous in ctx_sb row 64 ->
   one DMA to [128,8] for partition-parallel DVE reciprocal -> DMA to a
   DRAM bounce row -> stride-0 broadcast DMA (GPSIMD queue, see note
   above) to rep[64,1024] SBUF -> two GPSIMD multiplies write normalized
   bf16 ctx^T into CT. All off the PE's critical path (the gpsimd
   partition_broadcast ucode alternative costs 5.5 us and a PE
   outer-product alternative stalls the in-order PE stream).
 - Output projection + maxpool run per j-chunk, emission deferred ~20
   scheduler steps (34) so the normalize chains complete in the shadow of
   the next j PE work. bo is folded into the classifier bias on the host
   (max commutes with the per-feature constant bo).
"""

import sys

import numpy as np

if "/opt/trn_rl_repo" not in sys.path:
    sys.path.insert(0, "/opt/trn_rl_repo")

from collections import deque

import concourse.bass as bass
import concourse.bacc as bacc
import concourse.tile as tile
from concourse import mybir
from concourse.masks import make_identity

F32 = mybir.dt.float32
BF16 = mybir.dt.bfloat16
F8 = mybir.dt.float8e4
U8 = mybir.dt.uint8
I32 = mybir.dt.int32
ADD = mybir.AluOpType.add
MULT = mybir.AluOpType.mult
MAXOP = mybir.AluOpType.max
IDENT_FN = mybir.ActivationFunctionType.Identity
COPY_FN = mybir.ActivationFunctionType.Copy
DR = mybir.MatmulPerfMode.DoubleRow
X_AXIS = mybir.AxisListType.X

# exp(s/8) ~= bitcast_e4m3(u8(s * SCH_A + SCH_B)); the e4m3 bit pattern of
# exp(s/8) is affine in s (Schraudolph). Tuned in sim; robust to the
# (unknown) HW round-vs-truncate convert mode since the resulting global
# scale on P cancels in the softmax denominator.
SCH_A = 1.4426950408889634
SCH_B = 55.35

B = 8
E = 512
H = 8
DH = 64
OUT = 10
N_CORES = 8

# Exp engine per (j, head-pair) unit. GPSIMD cannot read PSUM on TRN2, so
# only ACT and DVE can run the exp-convert. One engine per unit keeps each
# softmax row on a single convert-rounding mode; the window-2 scheduler
# keeps both engines busy on alternating units. (A 9/7 ACT-heavy split
# was tried for busy-balance but the window-2 scheduler serializes the
# adjacent same-engine pair and costs ~11 us of span.)
ENG_PATTERN = ["act", "dve"] * 8


def build(S=2048, VOCAB=50257, pattern=None, tail_units=2,
          delay=34, window=2, m_yield=4, ptb=24, kbias_act=False):
    """Build the per-core Bass program (same program on all 8 cores)."""
    nc = bacc.Bacc()

    NT = S // 128   # 128-token tiles (16)
    NJ = S // 512   # 512-token q-chunks (4)
    NE = E // 128   # 128-feature chunks (4)
    NP = NT // 2    # pairs of token tiles for DoubleRow (8)

    xi = nc.declare_dram_parameter("xi", [128, NT], I32, isOutput=False)
    emb = nc.declare_dram_parameter("emb", [VOCAB, E], BF16, isOutput=False)
    wq = nc.declare_dram_parameter("wq", [E, E], BF16, isOutput=False)
    wk = nc.declare_dram_parameter("wk", [E, E], BF16, isOutput=False)
    wv = nc.declare_dram_parameter("wv", [E, E], BF16, isOutput=False)
    wo = nc.declare_dram_parameter("wo", [E, E], BF16, isOutput=False)
    wc = nc.declare_dram_parameter("wc", [E, OUT], F32, isOutput=False)
    bq = nc.declare_dram_parameter("bq", [128, NE], F32, isOutput=False)
    bk = nc.declare_dram_parameter("bk", [128, NE], F32, isOutput=False)
    bo = nc.declare_dram_parameter("bo", [128, NE], F32, isOutput=False)
    bv = nc.declare_dram_parameter("bv", [1, E], BF16, isOutput=False)
    bc = nc.declare_dram_parameter("bc", [OUT, 1], F32, isOutput=False)
    out = nc.declare_dram_parameter("out", [OUT, 1], F32, isOutput=True)

    with tile.TileContext(nc) as tc:
        with (
            tc.tile_pool(name="consts", bufs=1) as consts,
            tc.tile_pool(name="qkT", bufs=1) as qkT_pool,
            tc.tile_pool(name="va2p", bufs=1) as va2_pool,
            tc.tile_pool(name="ctxT", bufs=1) as ctxT_pool,
            tc.tile_pool(name="eTp", bufs=1) as eT_pool,
            tc.tile_pool(name="projw", bufs=1) as projw,
            tc.tile_pool(name="fin", bufs=1) as fin_pool,
        ):
            # ---- constants (emission order = DMA priority: index + QKV
            # weights first so the gather/projection pipeline starts ASAP)
            idx_sb = consts.tile([128, NT], I32, tag="idx")
            nc.sync.dma_start(out=idx_sb, in_=xi[:, :])
            wq_sb = [projw.tile([128, E], BF16, tag=f"wq{k}", name=f"wq{k}")
                     for k in range(NE)]
            wk_sb = [projw.tile([128, E], BF16, tag=f"wk{k}", name=f"wk{k}")
                     for k in range(NE)]
            wv_sb = [projw.tile([128, E], BF16, tag=f"wv{k}", name=f"wv{k}")
                     for k in range(NE)]
            for k in range(NE):
                nc.sync.dma_start(out=wv_sb[k], in_=wv[k * 128:(k + 1) * 128, :])
                nc.sync.dma_start(out=wk_sb[k], in_=wk[k * 128:(k + 1) * 128, :])
                nc.sync.dma_start(out=wq_sb[k], in_=wq[k * 128:(k + 1) * 128, :])
            ident = consts.tile([128, 128], BF16, tag="ident")
            make_identity(nc, ident)
            wo_sb = [consts.tile([128, E], BF16, tag=f"wo{k}", name=f"wo{k}")
                     for k in range(NE)]
            for k in range(NE):
                nc.sync.dma_start(out=wo_sb[k], in_=wo[k * 128:(k + 1) * 128, :])
            wc_sb = [consts.tile([128, OUT], F32, tag=f"wc{k}", name=f"wc{k}")
                     for k in range(NE)]
            for k in range(NE):
                nc.sync.dma_start(out=wc_sb[k], in_=wc[k * 128:(k + 1) * 128, :])
            bq_sb = consts.tile([128, NE], F32, tag="bq")
            nc.sync.dma_start(out=bq_sb, in_=bq[:, :])
            bk_sb = consts.tile([128, NE], F32, tag="bk")
            nc.sync.dma_start(out=bk_sb, in_=bk[:, :])
            bo_sb = consts.tile([128, NE], F32, tag="bo")
            nc.sync.dma_start(out=bo_sb, in_=bo[:, :])
            bv_sb = consts.tile([1, E], BF16, tag="bv")
            nc.sync.dma_start(out=bv_sb, in_=bv[:, :])
            bc_sb = consts.tile([OUT, 1], F32, tag="bc")
            nc.sync.dma_start(out=bc_sb, in_=bc[:, :])
            ones_row = consts.tile([1, 128], BF16, tag="ones")
            nc.vector.memset(ones_row, 1.0)
            # e^T as ONE tile [128, NE*S] bf16 (feature chunk kk at columns
            # kk*S..): lets each token tile evict with a single strided copy.
            eT = eT_pool.tile([128, NE * S], BF16, tag="eT", name="eT")

            # persistent activations
            QT = [qkT_pool.tile([128, S], BF16, tag=f"qt{k}", name=f"qt{k}")
                  for k in range(NE)]
            KT = [qkT_pool.tile([128, S], BF16, tag=f"kt{k}", name=f"kt{k}")
                  for k in range(NE)]
            # V fp8 pair tiles: [128 tok, H * (2 k-subtiles * 80)]; per head
            # two [V_h | 1 | pad] blocks at stride 80 (DoubleRow ldweights
            # requires subtile step % 16 == 0). Preset to 1.0 so the ones
            # columns stay; pad columns are never read.
            va2 = [va2_pool.tile([128, H * 160], F8, tag=f"va{p}",
                                 name=f"va{p}") for p in range(NP)]
            CT = [ctxT_pool.tile([128, S], BF16, tag=f"ct{k}", name=f"ct{k}")
                  for k in range(NE)]

            # ====== single fused phase: gathers upfront, everything ======
            # else (transposes, V/K/Q projections) is emitted lazily inside
            # the attention pipeline as unit prerequisites, so the first
            # scores run ~15 us after launch instead of ~35 us, and the PE
            # stays dense (warm p-state) throughout.
            with (
                tc.tile_pool(name="ptp", bufs=ptb) as pt_pool,
                tc.tile_pool(name="rep", bufs=2) as rep_pool,
                tc.tile_pool(name="enat", bufs=1) as enat_pool,
                tc.tile_pool(name="dsc", bufs=4, space="DRAM") as dram_pool,
                tc.tile_pool(name="sps", bufs=3, space="PSUM") as sps,
                tc.tile_pool(name="ctxps", bufs=2, space="PSUM") as ctxps,
            ):
                # NOTE: batching multiple token tiles into one indirect
                # DMA (3D out AP) was tried: descriptor generation goes
                # pathological (~86 us per transfer) and mis-writes SBUF.
                # One 2D gather per token tile it stays. All 16 issue
                # upfront on the GPSIMD queue into persistent e_nat tiles
                # (the Q7 prepare at ~1.1 us each is the serial limiter).
                e_nats = [enat_pool.tile([128, E], BF16, tag=f"en{t}",
                                         name=f"en{t}") for t in range(NT)]
                for t in range(NT):
                    nc.gpsimd.indirect_dma_start(
                        out=e_nats[t][:],
                        out_offset=None,
                        in_=emb[:, :],
                        in_offset=bass.IndirectOffsetOnAxis(
                            ap=idx_sb[:, t:t + 1], axis=0
                        ),
                    )
                    # va2 memsets run on DVE while it idles under the
                    # gather chain; each must precede its V eviction
                    if t % 2 == 1:
                        nc.vector.memset(va2[t // 2][:], 1.0)

                evict_rr = ["dve", "act"]
                emitted_chunk = set()
                emitted_v = set()

                def ensure_chunk(jj):
                    # transposes + eT evictions for the 4 token tiles of
                    # chunk jj (atomic emission: no yields, so a partially
                    # emitted chunk can never interleave wrong-side of a
                    # consumer in the in-order PE queue)
                    if jj in emitted_chunk:
                        return
                    emitted_chunk.add(jj)
                    for tt in range(4):
                        t = 4 * jj + tt
                        tp = sps.tile([128, 512], BF16, tag="s", name="tp")
                        for f in range(NE):
                            nc.tensor.transpose(
                                out=tp[:, f * 128:(f + 1) * 128],
                                in_=e_nats[t][:, f * 128:(f + 1) * 128],
                                identity=ident[:],
                            )
                        dst = eT[:].rearrange(
                            "p (f s) -> p f s",
                            s=S)[:, :, t * 128:(t + 1) * 128]
                        src = tp[:].rearrange("p (f c) -> p f c", c=128)
                        if evict_rr[t % 2] == "dve":
                            nc.vector.tensor_copy(out=dst, in_=src)
                        else:
                            nc.scalar.copy(out=dst, in_=src)

                def ensure_v(p):
                    # V projection (token-major, +bv via ones-row matmul)
                    # -> fp8 pair tile va2[p], for the pair's two tiles
                    if p in emitted_v:
                        return
                    emitted_v.add(p)
                    for t in (2 * p, 2 * p + 1):
                        ps = sps.tile([128, 512], F32, tag="s", name="vps")
                        for kk in range(NE):
                            nc.tensor.matmul(
                                out=ps[:],
                                lhsT=eT[:, kk * S + t * 128:
                                        kk * S + (t + 1) * 128],
                                rhs=wv_sb[kk][:],
                                start=(kk == 0),
                                stop=False,
                            )
                        nc.tensor.matmul(
                            out=ps[:], lhsT=ones_row[:], rhs=bv_sb[:],
                            start=False, stop=True,
                        )
                        half = t % 2
                        vdst = va2[p][:].rearrange(
                            "p (h two c) -> p h two c", two=2, c=80)
                        nc.scalar.copy(
                            out=vdst[:, :, half, 0:DH],
                            in_=ps[:].rearrange("p (h c) -> p h c", c=DH),
                        )
                pooled = [fin_pool.tile([128, 1], F32, tag=f"pool{m}",
                                        name=f"pool{m}") for m in range(NE)]

                def emit_exp(eng, dst_u8, src):
                    if eng == "act":
                        nc.scalar.activation(
                            out=dst_u8, in_=src, func=COPY_FN,
                            scale=SCH_A, bias=SCH_B,
                        )
                    elif eng == "dve":
                        nc.vector.tensor_scalar(
                            out=dst_u8, in0=src,
                            scalar1=SCH_A, scalar2=SCH_B, op0=MULT, op1=ADD,
                        )
                    else:
                        nc.gpsimd.tensor_scalar(
                            out=dst_u8, in0=src,
                            scalar1=SCH_A, scalar2=SCH_B, op0=MULT, op1=ADD,
                        )

                # Q/K projection blocks are emitted lazily inside phase C,
                # right before the first unit that needs them — their PE
                # work fills exp-bound pipeline slack. They borrow sps slots.
                emitted_k = set()
                emitted_q = set()

                def emit_k_block(m, jj):
                    ps = sps.tile([128, 1024], F32, tag="s", name="kps")
                    for kk in range(NE):
                        nc.tensor.matmul(
                            out=ps[:, 0:512],
                            lhsT=wk_sb[kk][:, m * 128:(m + 1) * 128],
                            rhs=eT[:, kk * S + jj * 512:kk * S + (jj + 1) * 512],
                            start=(kk == 0),
                            stop=(kk == NE - 1),
                        )
                    if kbias_act:
                        nc.scalar.activation(
                            out=KT[m][:, jj * 512:(jj + 1) * 512],
                            in_=ps[:, 0:512], func=IDENT_FN,
                            bias=bk_sb[:, m:m + 1], scale=1.0,
                        )
                    else:
                        nc.vector.tensor_scalar_add(
                            out=KT[m][:, jj * 512:(jj + 1) * 512],
                            in0=ps[:, 0:512], scalar1=bk_sb[:, m:m + 1],
                        )

                def emit_q_block(m, jj):
                    ps = sps.tile([128, 1024], F32, tag="s", name="qps")
                    for kk in range(NE):
                        nc.tensor.matmul(
                            out=ps[:, 0:512],
                            lhsT=wq_sb[kk][:, m * 128:(m + 1) * 128],
                            rhs=eT[:, kk * S + jj * 512:kk * S + (jj + 1) * 512],
                            start=(kk == 0),
                            stop=(kk == NE - 1),
                        )
                    nc.scalar.activation(
                        out=QT[m][:, jj * 512:(jj + 1) * 512],
                        in_=ps[:, 0:512], func=IDENT_FN,
                        bias=bq_sb[:, m:m + 1], scale=1.0,
                    )

                def ensure_k(hp, jj):
                    if (hp, jj) in emitted_k:
                        return
                    emitted_k.add((hp, jj))
                    emit_k_block(hp, jj)

                def ensure_q(j, hp):
                    if (j, hp) in emitted_q:
                        return
                    emitted_q.add((j, hp))
                    emit_q_block(hp, j)

                def pv_matmul(ctx, p, h, ptf8, start, stop):
                    nc.tensor.matmul(
                        out=ctx[:],
                        lhsT=va2[p][:, h * 160:(h + 1) * 160]
                        .rearrange("p (two c) -> p two c", c=80)[:, :, 0:65],
                        rhs=ptf8,
                        start=start, stop=stop,
                        perf_mode=DR,
                        skip_group_check=True,
                    )

                def unit(j, hp, eng):
                    """One (j, head-pair) attention unit; yields per chunk.

                    Two-pass over heads: the chunk loop accumulates only
                    head-e (1 PSUM bank); the stored SBUF pt tiles are then
                    replayed through 8 more DR matmuls for head-o. This
                    halves live ctx banks, buying a third stile slot so
                    scores(k+1) overlaps exp(k).
                    """
                    u_idx = j * (H // 2) + hp
                    tail = u_idx >= 16 - tail_units
                    ensure_chunk(j)
                    ensure_q(j, hp)
                    yield
                    ctx_e = ctxps.tile([DH + 1, 512], F32, tag="ctx",
                                       name="ctx_e")
                    pts = []
                    for p in range(NP):
                        ensure_chunk(p // 2)
                        ensure_k(hp, p // 2)
                        ensure_v(p)
                        # pt layout [128, (k-subtile, head, q)] = the raw
                        # concatenation of the pair's two exp outputs; each
                        # head's two k-subtile P blocks sit at stride 1024,
                        # which DoubleRow accepts (step % 16 == 0).
                        pt = pt_pool.tile([128, 2048], U8, tag="pt",
                                          name="pt")
                        pts.append(pt)
                        ptf8 = pt.bitcast(F8).rearrange(
                            "p (two c) -> p two c", c=1024)
                        for half in range(2):
                            i = 2 * p + half
                            stile = sps.tile([128, 1024], F32, tag="s",
                                             name="stile")
                            nc.tensor.matmul(
                                out=stile[:, 0:512],
                                lhsT=KT[hp][0:64, i * 128:(i + 1) * 128],
                                rhs=QT[hp][0:64, j * 512:(j + 1) * 512],
                                start=True, stop=True,
                                tile_position=(0, 0),
                            )
                            nc.tensor.matmul(
                                out=stile[:, 512:1024],
                                lhsT=KT[hp][64:128, i * 128:(i + 1) * 128],
                                rhs=QT[hp][64:128, j * 512:(j + 1) * 512],
                                start=True, stop=True,
                                tile_position=(64, 0),
                            )
                            emit_exp(eng,
                                     pt[:, half * 1024:(half + 1) * 1024],
                                     stile[:])
                            if half == 1:
                                pv_matmul(ctx_e, p, 2 * hp,
                                          ptf8[:, :, 0:512],
                                          start=(p == 0), stop=(p == NP - 1))
                            yield
                    # evict head-e ctx on this unit's own exp engine (its
                    # queue has no pending work => no priority inversion),
                    # freeing the bank for the next unit.
                    ctx_sb = rep_pool.tile([DH + 1, 1024], F32, tag="ctxsb",
                                           bufs=4)
                    if eng == "act":
                        nc.scalar.copy(out=ctx_sb[:, 0:512], in_=ctx_e[:])
                    else:
                        nc.vector.tensor_copy(out=ctx_sb[:, 0:512],
                                              in_=ctx_e[:])
                    yield
                    # head-o replay from the stored pt tiles (pure PE burst)
                    ctx_o = ctxps.tile([DH + 1, 512], F32, tag="ctx",
                                       name="ctx_o")
                    for p in range(NP):
                        ptf8 = pts[p].bitcast(F8).rearrange(
                            "p (two c) -> p two c", c=1024)
                        pv_matmul(ctx_o, p, 2 * hp + 1,
                                  ptf8[:, :, 512:1024],
                                  start=(p == 0), stop=(p == NP - 1))
                        if p % 4 == 3:
                            yield
                    if eng == "act":
                        nc.scalar.copy(out=ctx_sb[:, 512:1024], in_=ctx_o[:])
                    else:
                        nc.vector.tensor_copy(out=ctx_sb[:, 512:1024],
                                              in_=ctx_o[:])
                    if tail:
                        # Last two units: exp work is done, so the
                        # normalize multiplies run on now-idle DVE instead
                        # of serializing on GPSIMD. (A [1,1024]
                        # single-partition reciprocal was tried to skip the
                        # l128 round-trip: 6 us on HW — partition-parallel
                        # [128,8] it stays.)
                        l128 = rep_pool.tile([128, 8], F32, tag="l128",
                                             bufs=4)
                        nc.sync.dma_start(out=l128[:],
                                          in_=ctx_sb[DH:DH + 1, :])
                        nc.vector.reciprocal(out=l128[:], in_=l128[:])
                        rrow_d = dram_pool.tile([1, 1024], F32, tag="rrowd",
                                                bufs=4)
                        nc.sync.dma_start(out=rrow_d[:], in_=l128[:])
                        rep = rep_pool.tile([64, 1024], F32, tag="rep",
                                            bufs=4)
                        nc.gpsimd.dma_start(
                            out=rep[:],
                            in_=rrow_d[:].to_broadcast([64, 1024]))
                        for off, h in ((0, 2 * hp), (512, 2 * hp + 1)):
                            nc.vector.tensor_tensor(
                                out=CT[hp][(h % 2) * 64:(h % 2) * 64 + 64,
                                           j * 512:(j + 1) * 512],
                                in0=ctx_sb[0:DH, off:off + 512],
                                in1=rep[:, off:off + 512],
                                op=MULT,
                            )
                        yield
                        return
                    # both denominator rows sit contiguous in ctx_sb row 64:
                    # ONE DMA round-trip feeds the partition-parallel
                    # reciprocal (bf16 out, feeds the PE broadcast).
                    l128 = rep_pool.tile([128, 8], F32, tag="l128", bufs=4)
                    nc.sync.dma_start(out=l128[:], in_=ctx_sb[DH:DH + 1, :])
                    nc.vector.reciprocal(out=l128[:], in_=l128[:])
                    rrow_d = dram_pool.tile([1, 1024], F32, tag="rrowd",
                                            bufs=4)
                    nc.sync.dma_start(out=rrow_d[:], in_=l128[:])
                    rep = rep_pool.tile([64, 1024], F32, tag="rep", bufs=4)
                    # stride-0 broadcast DMA from the DRAM bounce; read is
                    # issued from the GPSIMD queue (write on sync queue), so
                    # ordering runs through an explicit cross-queue
                    # completion semaphore rather than same-queue order.
                    nc.gpsimd.dma_start(out=rep[:],
                                        in_=rrow_d[:].to_broadcast([64, 1024]))
                    for off, h in ((0, 2 * hp), (512, 2 * hp + 1)):
                        nc.gpsimd.tensor_tensor(
                            out=CT[hp][(h % 2) * 64:(h % 2) * 64 + 64,
                                       j * 512:(j + 1) * 512],
                            in0=ctx_sb[0:DH, off:off + 512],
                            in1=rep[:, off:off + 512],
                            op=MULT,
                        )
                    yield

                def emit_outproj_m(j, m):
                    ps = sps.tile([128, 1024], F32, tag="s", name="ovps")
                    for kk in range(NE):
                        nc.tensor.matmul(
                            out=ps[:, 0:512],
                            lhsT=wo_sb[kk][:, m * 128:(m + 1) * 128],
                            rhs=CT[kk][:, j * 512:(j + 1) * 512],
                            start=(kk == 0),
                            stop=(kk == NE - 1),
                        )
                    if j == 0:
                        nc.vector.reduce_max(
                            out=pooled[m][:], in_=ps[:, 0:512], axis=X_AXIS,
                        )
                    else:
                        tmp = rep_pool.tile([128, 1], F32, tag="tmp")
                        nc.vector.reduce_max(
                            out=tmp[:], in_=ps[:, 0:512], axis=X_AXIS,
                        )
                        nc.vector.tensor_tensor(
                            out=pooled[m][:], in0=pooled[m][:],
                            in1=tmp[:], op=MAXOP,
                        )

                def outproj_gen(j, delay):
                    # Deferred so the next j's PE work is already queued in
                    # front of these CT-dependent matmuls — the normalize
                    # chains complete in the shadow of that work instead of
                    # stalling the in-order PE stream.
                    for _ in range(delay):
                        yield
                    for m in range(NE):
                        emit_outproj_m(j, m)
                        for _ in range(m_yield):
                            yield

                # software-pipelined unit scheduler: 2 units in flight
                gens = deque()
                for j in range(NJ):
                    for hp in range(H // 2):
                        u = j * (H // 2) + hp
                        eng = (pattern or ENG_PATTERN)[u]
                        gens.append((j, unit(j, hp, eng)))
                active = []
                pending_ops = []
                done_j = {j: 0 for j in range(NJ)}

                def pump_ops():
                    for g in list(pending_ops):
                        try:
                            next(g)
                        except StopIteration:
                            pending_ops.remove(g)

                while gens or active or pending_ops:
                    while len(active) < window and gens:
                        active.append(gens.popleft())
                    if not active:
                        pump_ops()
                        continue
                    j0, g = active.pop(0)
                    try:
                        next(g)
                        active.append((j0, g))
                    except StopIteration:
                        done_j[j0] += 1
                        if done_j[j0] == H // 2:
                            pending_ops.append(outproj_gen(j0, delay=delay))
                    pump_ops()

                # classifier: bo is folded into bc on the host
                # (logits = max(ctx@Wo)@Wc + (bo@Wc + bc)).
                cls = sps.tile([128, 1024], F32, tag="s", name="clsps")
                for kk in range(NE):
                    nc.tensor.matmul(
                        out=cls[0:OUT, 0:1],
                        lhsT=wc_sb[kk][:],
                        rhs=pooled[kk][:],
                        start=(kk == 0),
                        stop=(kk == NE - 1),
                    )
                logits = fin_pool.tile([OUT, 1], F32, tag="logits")
                nc.vector.tensor_scalar_add(
                    out=logits[:], in0=cls[0:OUT, 0:1], scalar1=bc_sb[:]
                )
                nc.sync.dma_start(out=out[:, :], in_=logits[:])

    nc.finalize()
    return nc


def make_in_maps(inputs):
    """Shard the full inputs into per-core (per-batch-row) input dicts."""
    import ml_dtypes

    bf16 = ml_dtypes.bfloat16
    x = np.asarray(inputs["x"]).astype(np.int32)          # [B, S]
    S = x.shape[1]
    emb = np.ascontiguousarray(
        np.asarray(inputs["emb_table"], dtype=np.float32).astype(bf16))
    shared = {
        "emb": emb,
        "wq": np.ascontiguousarray(np.asarray(inputs["Wq"]).astype(bf16)),
        "wk": np.ascontiguousarray(np.asarray(inputs["Wk"]).astype(bf16)),
        "wv": np.ascontiguousarray(np.asarray(inputs["Wv"]).astype(bf16)),
        "wo": np.ascontiguousarray(np.asarray(inputs["Wo"]).astype(bf16)),
        "wc": np.ascontiguousarray(np.asarray(inputs["Wc"], dtype=np.float32)),
        "bq": np.ascontiguousarray(
            np.asarray(inputs["bq"], dtype=np.float32).reshape(4, 128).T),
        "bk": np.ascontiguousarray(
            np.asarray(inputs["bk"], dtype=np.float32).reshape(4, 128).T),
        "bo": np.ascontiguousarray(
            np.asarray(inputs["bo"], dtype=np.float32).reshape(4, 128).T),
        # bo folded into the classifier bias: logits = pooled@Wc + (bo@Wc+bc)
        "bc": np.ascontiguousarray(
            (np.asarray(inputs["bo"], dtype=np.float32)
             @ np.asarray(inputs["Wc"], dtype=np.float32)
             + np.asarray(inputs["bc"], dtype=np.float32)).reshape(OUT, 1)),
        "bv": np.ascontiguousarray(
            np.asarray(inputs["bv"]).astype(bf16).reshape(1, E)),
    }
    in_maps = []
    for c in range(x.shape[0]):
        xi = np.ascontiguousarray(x[c].reshape(S // 128, 128).T)  # [128, NT]
        in_maps.append({"xi": xi, **shared})
    return in_maps


_NC_CACHE = {}


def get_nc(S=2048, VOCAB=50257):
    key = (S, VOCAB)
    if key not in _NC_CACHE:
        _NC_CACHE[key] = build(S, VOCAB)
    return _NC_CACHE[key]


def run(inputs, trace=False):
    from concourse.bass_utils import run_bass_kernel_spmd

    nc = get_nc()
    in_maps = make_in_maps(inputs)
    res = run_bass_kernel_spmd(
        nc, in_maps, list(range(N_CORES)), trace=trace
    )
    outs = np.stack(
        [res.results[c]["out"].reshape(OUT) for c in range(N_CORES)]
    ).astype(np.float32)
    return outs, res


def kernel(**inputs):
    outs, _ = run(inputs, trace=False)
    return outs

